# revision 8
# baseline (speedup 1.0000x reference)
"""GRU (Keras reset_after=True, relu candidate) Trainium2 Bass kernel.

Problem shapes (hardcoded): B=256, T=128, F=512, H=512, 3H=1536.
Sharding: data-parallel over batch across 8 NeuronCores (32 batch each),
params replicated. Everything on-device; host only reshapes/casts shards.

Device-side design (per core, b=32 local batch):
  - Transposed layout everywhere: state h kept as hT[p, k, b] (H on
    partitions in 4 chunks of 128; batch b=32 on the free dim) so that all
    gate elementwise work runs with 128 active partitions and tiny free dims.
  - Phase 1 (projection): xp = x @ kernel + bias, computed at full PE rate
    with float32r (moving N=512), output stored transposed per-step in a
    DRAM scratch buffer xpq[t, j, p, b] (j indexes 12 chunks of the 3H dim).
  - Phase 2 (recurrence, T sequential steps): rec.T = rec_kernel.T-chunks
    (stationary, bf16 => fast weight load) x hT (moving, 32 cols). 48
    weight chunks of [128,128] per step accumulate into 3 PSUM tiles
    (r-gate, h-gate, z-gate; separate banks so gates can read while PE
    writes the next group). Gates on DVE + ACT (sigmoid), relu via DVE max.
  - Head: y = hT . Wd + bd via 4 accumulating matmuls into a [1, 32] PSUM.
"""

import numpy as np
import ml_dtypes

import concourse.bass as bass
import concourse.mybir as mybir
import concourse.tile as tile
from concourse import bass_utils

B, T, F, H = 256, 128, 512, 512
NC = 8
BL = B // NC          # 32 local batch
KF = F // 128         # 4 chunks of input feature dim
KH = H // 128         # 4 chunks of hidden dim
NJ = 3 * H // 128     # 12 chunks of the 3H gate dim
F32 = mybir.dt.float32
F32R = mybir.dt.float32r
BF16 = mybir.dt.bfloat16


def _split_excess_waits(nc, max_waits=1):
    """This container's walrus only accepts 1 sync-wait command per
    instruction; move excess waits onto preceding same-engine NOPs."""
    for f in nc.m.functions:
        for blk in f.blocks:
            new_list = []
            changed = False
            for inst in blk.instructions:
                si = inst.sync_info
                if si is not None and si.on_wait and len(si.on_wait) > max_waits:
                    waits = list(si.on_wait)
                    head, keep = waits[:-max_waits], waits[-max_waits:]
                    for ci in range(0, len(head), max_waits):
                        new_list.append(mybir.InstNoOp(
                            name=f"{inst.name}-wsplit-{ci}",
                            engine=inst.engine,
                            ins=[], outs=[],
                            sync_info=mybir.SyncInfo(
                                on_wait=head[ci:ci + max_waits], on_update=[]),
                        ))
                    si.on_wait = keep
                    inst.sync_info = si
                    changed = True
                new_list.append(inst)
            if changed:
                blk.instructions = new_list
    return nc


def build_program(n_steps=T, has_brh=False):
    nc = bass.Bass()

    xT = nc.dram_tensor("xT", [KF, 128, n_steps * BL], F32R, kind="ExternalInput")
    ker = nc.dram_tensor("ker", [KF, 128, 3 * H], F32R, kind="ExternalInput")
    recK = nc.dram_tensor("recK", [KH, 128, 3 * H], BF16, kind="ExternalInput")
    bT = nc.dram_tensor("bT", [128, NJ], F32, kind="ExternalInput")
    brh = nc.dram_tensor("brh", [128, KH], F32, kind="ExternalInput")
    wdT = nc.dram_tensor("wdT", [KH, 128, 1], F32, kind="ExternalInput")
    bdv = nc.dram_tensor("bdv", [1, 1], F32, kind="ExternalInput")
    y = nc.dram_tensor("y", [1, BL], F32, kind="ExternalOutput")

    # column-chunks of the projection moving dim (t*BL+b), up to 512 wide
    M = n_steps * BL
    CW = min(512, M)            # chunk width (512 => 16 steps per chunk)
    n_cc = (M + CW - 1) // CW
    TC = CW // BL               # steps per column-chunk

    with tile.TileContext(nc) as tc:
        with (
            tc.tile_pool(name="persist", bufs=1) as persist,
            tc.tile_pool(name="state", bufs=1) as state,
            tc.tile_pool(name="dram", bufs=1, space="DRAM") as dpool,
        ):
            # --- load replicated params to SBUF
            recK_sb = persist.tile([128, KH, 3 * H], BF16)
            nc.sync.dma_start(out=recK_sb[:], in_=recK[:].rearrange("k p n -> p k n"))
            bT_sb = persist.tile([128, NJ], F32)
            nc.sync.dma_start(out=bT_sb[:], in_=bT[:])
            brh_sb = persist.tile([128, KH], F32)
            nc.sync.dma_start(out=brh_sb[:], in_=brh[:])
            wd_sb = persist.tile([128, KH, 1], F32)
            nc.sync.dma_start(out=wd_sb[:], in_=wdT[:].rearrange("k p o -> p k o"))
            bd_sb = persist.tile([1, 1], F32)
            nc.sync.dma_start(out=bd_sb[:], in_=bdv[:])

            xpq = dpool.tile([n_steps, NJ, 128, BL], F32)

            # ---------------- Phase 1: input projection ----------------
            with (
                tc.tile_pool(name="proj_in", bufs=1) as proj_in,
                tc.tile_pool(name="proj_ps", bufs=8, space="PSUM") as proj_ps,
                tc.tile_pool(name="proj_out", bufs=6) as proj_out,
            ):
                ker_sb = proj_in.tile([128, KF, 3 * H], F32R)
                nc.sync.dma_start(out=ker_sb[:], in_=ker[:].rearrange("k p n -> p k n"))
                xsb = proj_in.tile([128, KF, n_steps * BL], F32R)
                nc.sync.dma_start(out=xsb[:], in_=xT[:].rearrange("k p m -> p k m"))

                for j in range(NJ):
                    for cg in range(0, n_cc, 4):
                        cs = list(range(cg, min(cg + 4, n_cc)))
                        pts = [proj_ps.tile([128, CW], F32, name="proj_pt",
                                            tag="proj_pt") for _ in cs]
                        for kf in range(KF):
                            for pt, c in zip(pts, cs):
                                nc.tensor.matmul(
                                    pt[:],
                                    lhsT=ker_sb[:, kf, 128 * j:128 * (j + 1)],
                                    rhs=xsb[:, kf, CW * c:CW * (c + 1)],
                                    start=(kf == 0), stop=(kf == KF - 1),
                                )
                        for i, (pt, c) in enumerate(zip(pts, cs)):
                            xq_sb = proj_out.tile([128, CW], F32)
                            if i % 2 == 0:
                                nc.vector.tensor_scalar_add(
                                    xq_sb[:], pt[:], bT_sb[:, j:j + 1])
                            else:
                                nc.scalar.activation(
                                    xq_sb[:], pt[:],
                                    mybir.ActivationFunctionType.Identity,
                                    bias=bT_sb[:, j:j + 1])
                            nc.sync.dma_start(
                                out=xpq[TC * c:TC * (c + 1), j, :, :]
                                    .rearrange("t p b -> p t b"),
                                in_=xq_sb[:].rearrange("p (t b) -> p t b", b=BL),
                            )

            # ---------------- Phase 2: recurrence ----------------
            h32 = state.tile([128, KH, BL], F32)
            hbf = state.tile([128, KH, BL], BF16)
            nc.vector.memset(h32[:], 0.0)
            nc.vector.memset(hbf[:], 0.0)

            with (
                tc.tile_pool(name="xq", bufs=4) as xq_pool,
                tc.tile_pool(name="ps", bufs=2, space="PSUM") as ps_pool,
                tc.tile_pool(name="gates", bufs=2) as gates,
            ):
                for t in range(n_steps):
                    xq_t = xq_pool.tile([128, NJ, BL], F32)
                    nc.sync.dma_start(
                        out=xq_t[:], in_=xpq[t].rearrange("j p b -> p j b"))

                    ps_r = ps_pool.tile([128, KH, BL], F32, tag="ps_r")
                    ps_h = ps_pool.tile([128, KH, BL], F32, tag="ps_h")
                    ps_z = ps_pool.tile([128, KH, BL], F32, tag="ps_z")
                    # j order: r gates (4..7), h gates (8..11), z gates (0..3)
                    for ps_x, j0 in ((ps_r, 4), (ps_h, 8), (ps_z, 0)):
                        for jj in range(KH):
                            j = j0 + jj
                            for k in range(KH):
                                nc.tensor.matmul(
                                    ps_x[:, jj, :],
                                    lhsT=recK_sb[:, k, 128 * j:128 * (j + 1)],
                                    rhs=hbf[:, k, :],
                                    start=(k == 0), stop=(k == KH - 1),
                                )

                    pre_r = gates.tile([128, KH, BL], F32, tag="pre_r")
                    nc.vector.tensor_add(pre_r[:], ps_r[:], xq_t[:, 4:8, :])
                    r_g = gates.tile([128, KH, BL], F32, tag="r_g")
                    nc.scalar.activation(
                        r_g[:], pre_r[:], mybir.ActivationFunctionType.Sigmoid)

                    if has_brh:
                        rh_sb = gates.tile([128, KH, BL], F32, tag="rh")
                        bb = brh_sb[:, :]
                        brh_bc = bass.AP(
                            tensor=bb.tensor, offset=bb.offset,
                            ap=[bb.ap[0], bb.ap[1], [0, BL]])
                        nc.vector.tensor_add(rh_sb[:], ps_h[:], brh_bc)
                        rh_src = rh_sb[:]
                    else:
                        rh_src = ps_h[:]
                    hh = gates.tile([128, KH, BL], F32, tag="hh")
                    nc.vector.tensor_mul(hh[:], r_g[:], rh_src)
                    nc.vector.tensor_add(hh[:], hh[:], xq_t[:, 8:12, :])
                    nc.vector.tensor_scalar_max(hh[:], hh[:], 0.0)

                    pre_z = gates.tile([128, KH, BL], F32, tag="pre_z")
                    nc.vector.tensor_add(pre_z[:], ps_z[:], xq_t[:, 0:4, :])
                    z_g = gates.tile([128, KH, BL], F32, tag="z_g")
                    nc.scalar.activation(
                        z_g[:], pre_z[:], mybir.ActivationFunctionType.Sigmoid)

                    d_t = gates.tile([128, KH, BL], F32, tag="d_t")
                    nc.vector.tensor_sub(d_t[:], h32[:], hh[:])
                    nc.vector.tensor_mul(d_t[:], z_g[:], d_t[:])
                    nc.vector.tensor_add(h32[:], d_t[:], hh[:])
                    nc.vector.tensor_copy(out=hbf[:], in_=h32[:])

                # ---------------- head: y = h . Wd + bd ----------------
                psy = ps_pool.tile([1, BL], F32, tag="psy")
                for k in range(KH):
                    nc.tensor.matmul(
                        psy[:], lhsT=wd_sb[:, k, :], rhs=h32[:, k, :],
                        start=(k == 0), stop=(k == KH - 1),
                    )
                y_sb = gates.tile([1, BL], F32, tag="y_sb")
                nc.vector.tensor_scalar_add(y_sb[:], psy[:], bd_sb[0:1, 0:1])
                nc.sync.dma_start(out=y[:], in_=y_sb[:])

    return nc


def _prep_inputs(x, kernel, rec_kernel, bias, Wd, bd, n_steps=T):
    """Host-side: shard + lay out per-core input arrays."""
    x = np.asarray(x, np.float32)
    kernel = np.asarray(kernel, np.float32)
    rec_kernel = np.asarray(rec_kernel, np.float32)
    bias = np.asarray(bias, np.float32)
    Wd = np.asarray(Wd, np.float32)
    bd = np.asarray(bd, np.float32)

    ker_a = np.ascontiguousarray(kernel.reshape(KF, 128, 3 * H))
    recK_a = np.ascontiguousarray(
        rec_kernel.reshape(KH, 128, 3 * H).astype(ml_dtypes.bfloat16))
    bfull = bias[0].copy()
    bfull[:2 * H] += bias[1][:2 * H]
    bT_a = np.ascontiguousarray(bfull.reshape(NJ, 128).T)
    brh_a = np.ascontiguousarray(bias[1][2 * H:].reshape(KH, 128).T)
    wdT_a = np.ascontiguousarray(Wd.reshape(KH, 128, 1))
    bdv_a = bd.reshape(1, 1)

    in_maps = []
    for c in range(NC):
        xc = x[BL * c:BL * (c + 1), :n_steps]          # [32, T, 512]
        xT_c = np.ascontiguousarray(
            xc.transpose(2, 1, 0).reshape(KF, 128, n_steps * BL))
        in_maps.append({
            "xT": xT_c, "ker": ker_a, "recK": recK_a, "bT": bT_a,
            "brh": brh_a, "wdT": wdT_a, "bdv": bdv_a,
        })
    return in_maps, bool(np.any(brh_a))


_cache = {}


def run(inputs, n_steps=T, trace=False, trace_kwargs=None):
    in_maps, has_brh = _prep_inputs(
        inputs["x"], inputs["kernel"], inputs["rec_kernel"],
        inputs["bias"], inputs["Wd"], inputs["bd"], n_steps=n_steps)
    key = (n_steps, has_brh)
    if key not in _cache:
        _cache[key] = _split_excess_waits(
            build_program(n_steps=n_steps, has_brh=has_brh))
    nc = _cache[key]
    kw = {}
    if trace:
        kw.update(trace=True, trace_cores=[0])
        if trace_kwargs:
            kw.update(trace_kwargs=trace_kwargs)
    try:
        res = bass_utils.run_bass_kernel_spmd(
            nc, in_maps, core_ids=list(range(NC)), **kw)
    except ModuleNotFoundError:
        # no axon NTFF profiling hook in this container
        res = bass_utils.run_bass_kernel_spmd(
            nc, in_maps, core_ids=list(range(NC)))
    out = np.empty((NC * BL, 1), np.float32)
    for c in range(NC):
        out[BL * c:BL * (c + 1), 0] = res.results[c]["y"][0]
    return out, res


def kernel(x, kernel, rec_kernel, bias, Wd, bd):
    out, _ = run({"x": x, "kernel": kernel, "rec_kernel": rec_kernel,
                  "bias": bias, "Wd": Wd, "bd": bd})
    return out
